# revision 1
# baseline (speedup 1.0000x reference)
# Multi-head causal self-attention with RoPE on 8 NeuronCores (Trainium2).
#
# Sharding: zero-communication data parallel. Core c handles batch b = c//2
# and a balanced half of that batch's queries (half = c%2):
#   half 0 -> query rows [0:512) u [1536:2048)   (early + late stripe)
#   half 1 -> query rows [512:1536)
# Both halves attend over the full 2048-token K/V of their batch (K/V
# projection is duplicated across the pair — the price of zero comms).
# Every core runs the same program (SPMD); per-core differences live purely
# in the input data (gathered query slices, RoPE tables, causal masks).
#
# Layouts (on chip, bf16 compute / f32 accumulate):
#   q^T, k^T  [128 part = head-pair dims, tokens]   d-major for S^T matmuls
#   V         [128 part = tokens, dims]             token-major, +ones col for
#                                                   the softmax denominator
#   S^T tiles [128 j-tokens, 512 queries]           softmax along PARTITION j
#                                                   via matmul-with-ones —
#                                                   no transposes anywhere.
# RoPE uses an "evens-then-odds" permuted head layout (baked into Wq/Wk
# columns host-side) so the rotation partner is a fixed +-32 partition shift.

import sys

import numpy as np
import ml_dtypes

for _p in ("/opt/trn_rl_repo",):
    try:
        import concourse.bass  # noqa: F401
        break
    except ImportError:
        sys.path.insert(0, _p)

import concourse.bass as bass
import concourse.tile as tile
from concourse import mybir
from concourse.bass_utils import run_bass_kernel_spmd

B, T, D, H, DH = 4, 2048, 1024, 16, 64
THETA = 10000.0
NCORES = 8
P = 128
OC = 8    # 128-wide output-dim chunks (head pairs)
DC = 8    # 128-wide input-dim chunks
NQ = 1024  # queries per core
BLK = 512  # query block width
JA, JB = 8, 16  # j-tiles (128 kv tokens each) per block A / B

f32 = mybir.dt.float32
bf16 = mybir.dt.bfloat16
BF = ml_dtypes.bfloat16


# ---------------------------------------------------------------- host prep

def _perm():
    """Column permutation: within each head's 64 dims, evens then odds."""
    p = np.empty(D, np.int64)
    for h in range(H):
        for m in range(32):
            p[h * 64 + m] = h * 64 + 2 * m
            p[h * 64 + 32 + m] = h * 64 + 2 * m + 1
    return p


def _qpos(core):
    half = core % 2
    if half == 0:
        return np.concatenate([np.arange(0, 512), np.arange(1536, 2048)])
    return np.arange(512, 1536)


def _rope_tables(pos):
    """cos/sin tables [128, len(pos)] for the permuted (evens-first) layout.

    Slot p within a 64-dim head: m = p % 64; freq index = m % 32; the
    rotation partner is p XOR 32 (within the head); sign of the sin term is
    -1 for m < 32, +1 for m >= 32.
    """
    inv = THETA ** (-(np.arange(0, DH, 2, dtype=np.float64) / DH))  # [32]
    m = np.arange(P) % 64
    fi = m % 32
    ang = pos[None, :].astype(np.float64) * inv[fi][:, None]  # [128, L]
    cos = np.cos(ang)
    sin = np.sin(ang) * np.where(m < 32, -1.0, 1.0)[:, None]
    return cos.astype(np.float32), sin.astype(np.float32)


def _masks(core):
    """maskA [8,128,512], maskB [8,128,512] (j-tiles 8..15) for this core."""
    qp = _qpos(core)
    qa, qb = qp[:BLK], qp[BLK:]
    jj = np.arange(P)
    mA = np.empty((JA, P, BLK), np.float32)
    for jt in range(JA):
        mA[jt] = ((jt * P + jj)[:, None] <= qa[None, :]).astype(np.float32)
    mB = np.empty((8, P, BLK), np.float32)
    for jt in range(8, 16):
        mB[jt - 8] = ((jt * P + jj)[:, None] <= qb[None, :]).astype(np.float32)
    return mA, mB


def host_prep(x, Wq, bq, Wk, bk, Wv, bv, Wo, bo):
    """Build the 8 per-core input dicts (numpy, bf16 unless noted)."""
    perm = _perm()
    WqT = np.ascontiguousarray(Wq.T[:, perm]).astype(BF)
    WkT = np.ascontiguousarray(Wk.T[:, perm]).astype(BF)
    WvT = np.ascontiguousarray(Wv.T).astype(BF)
    WoT = np.ascontiguousarray(Wo.T).astype(BF)
    bqp = bq[perm].reshape(1, D).astype(BF)
    bkp = bk[perm].reshape(1, D).astype(BF)
    bvp = bv.reshape(1, D).astype(BF)
    bop = bo.reshape(1, D).astype(BF)
    ck, sk = _rope_tables(np.arange(T))
    in_maps = []
    for c in range(NCORES):
        b = c // 2
        qp = _qpos(c)
        cq, sq = _rope_tables(qp)
        mA, mB = _masks(c)
        xb = x[b]  # [T, D]
        in_maps.append({
            "xT": np.ascontiguousarray(xb.T).astype(BF),
            "xqT": np.ascontiguousarray(xb[qp].T).astype(BF),
            "WqT": WqT, "WkT": WkT, "WvT": WvT, "WoT": WoT,
            "bq": bqp, "bk": bkp, "bv": bvp, "bo": bop,
            "cq": cq.astype(BF), "sq": sq.astype(BF),
            "ck": ck.astype(BF), "sk": sk.astype(BF),
            "mA": mA.astype(BF), "mB": mB.astype(BF),
        })
    return in_maps


def assemble(results):
    y = np.empty((B, T, D), np.float32)
    for c in range(NCORES):
        y[c // 2, _qpos(c), :] = results[c]["out"]
    return y


# ------------------------------------------------------------- device build

def _legalize_waits(nc, max_waits=1):
    """Limit every instruction to one sync-wait command.

    Walrus's per-instruction structs encode a single sync wait; Tile can
    emit more. For any instruction with k > 1 waits, insert k-1 nops on
    the same engine immediately before it, each carrying one wait —
    position-preserving, so semantics are unchanged.
    """
    eng_obj = {
        mybir.EngineType.PE: nc.tensor,
        mybir.EngineType.Activation: nc.scalar,
        mybir.EngineType.DVE: nc.vector,
        mybir.EngineType.Pool: nc.gpsimd,
        mybir.EngineType.SP: nc.sync,
    }
    fn = nc.m.functions[0]
    for blk in fn.blocks:
        insts = list(blk.instructions)
        new = []
        for inst in insts:
            si = inst.sync_info
            nw = len(si.on_wait) if si is not None else 0
            if nw > max_waits:
                for w in si.on_wait[: nw - max_waits]:
                    eng_obj[inst.engine].nop()
                    nop = fn.blocks[-1].instructions[-1]
                    fn.blocks[-1].instructions = \
                        fn.blocks[-1].instructions[:-1]
                    nop.sync_info = mybir.SyncInfo(on_wait=[w], on_update=[])
                    new.append(nop)
                inst.sync_info = mybir.SyncInfo(
                    on_wait=list(si.on_wait[nw - max_waits:]),
                    on_update=list(si.on_update))
            new.append(inst)
        blk.instructions = new


def build_nc(use_bias):
    from contextlib import ExitStack

    nc = bass.Bass("TRN2", target_bir_lowering=False, debug=False,
                   num_devices=NCORES)
    Exp = mybir.ActivationFunctionType.Exp

    xT = nc.dram_tensor("xT", [D, T], bf16, kind="ExternalInput").ap()
    xqT = nc.dram_tensor("xqT", [D, NQ], bf16, kind="ExternalInput").ap()
    WqT = nc.dram_tensor("WqT", [D, D], bf16, kind="ExternalInput").ap()
    WkT = nc.dram_tensor("WkT", [D, D], bf16, kind="ExternalInput").ap()
    WvT = nc.dram_tensor("WvT", [D, D], bf16, kind="ExternalInput").ap()
    WoT = nc.dram_tensor("WoT", [D, D], bf16, kind="ExternalInput").ap()
    if use_bias:
        bq_d = nc.dram_tensor("bq", [1, D], bf16, kind="ExternalInput").ap()
        bk_d = nc.dram_tensor("bk", [1, D], bf16, kind="ExternalInput").ap()
        bv_d = nc.dram_tensor("bv", [1, D], bf16, kind="ExternalInput").ap()
        bo_d = nc.dram_tensor("bo", [1, D], bf16, kind="ExternalInput").ap()
    cq_d = nc.dram_tensor("cq", [P, NQ], bf16, kind="ExternalInput").ap()
    sq_d = nc.dram_tensor("sq", [P, NQ], bf16, kind="ExternalInput").ap()
    ck_d = nc.dram_tensor("ck", [P, T], bf16, kind="ExternalInput").ap()
    sk_d = nc.dram_tensor("sk", [P, T], bf16, kind="ExternalInput").ap()
    mA_d = nc.dram_tensor("mA", [JA, P, BLK], bf16, kind="ExternalInput").ap()
    mB_d = nc.dram_tensor("mB", [8, P, BLK], bf16, kind="ExternalInput").ap()
    out_d = nc.dram_tensor("out", [NQ, D], f32, kind="ExternalOutput").ap()
    den_d = nc.dram_tensor("den_scratch", [32, BLK], f32)

    with tile.TileContext(nc) as tc, ExitStack() as ctx:
        big = ctx.enter_context(tc.tile_pool(name="big", bufs=1))
        const = ctx.enter_context(tc.tile_pool(name="const", bufs=1))
        ph1_stack = ExitStack()
        ph1 = ph1_stack.enter_context(tc.tile_pool(name="ph1", bufs=1))
        rpool = ph1_stack.enter_context(tc.tile_pool(name="rp", bufs=2))
        psmm = ph1_stack.enter_context(
            tc.tile_pool(name="psmm", bufs=3, space="PSUM"))

        # ---- persistent SBUF tensors (per-dc tiles so compute starts
        # as soon as the first chunks land)
        def load_rows(src, L, tagp):
            tiles = []
            for dc in range(DC):
                t = ph1.tile([P, L], bf16, tag=f"{tagp}{dc}")
                nc.sync.dma_start(t, src[dc * P:(dc + 1) * P, :])
                tiles.append(t)
            return tiles
        x_s = load_rows(xT, T, "x_s")
        xq_s = load_rows(xqT, NQ, "xq_s")
        wv_s = load_rows(WvT, D, "wv_s")
        wq_s = load_rows(WqT, D, "wq_s")
        wk_s = load_rows(WkT, D, "wk_s")
        qfin = big.tile([P, OC, NQ], bf16, tag="qfin")
        kfin = big.tile([P, OC, T], bf16, tag="kfin")
        vaug = big.tile([P, 16, H, 65], bf16, tag="vaug")
        nc.vector.memset(vaug[:, :, :, 64:65], 1.0)

        cq_s = const.tile([P, NQ], bf16, tag="cq")
        nc.sync.dma_start(cq_s, cq_d)
        sq_s = const.tile([P, NQ], bf16, tag="sq")
        nc.sync.dma_start(sq_s, sq_d)
        ck_s = const.tile([P, T], bf16, tag="ck")
        nc.sync.dma_start(ck_s, ck_d)
        sk_s = const.tile([P, T], bf16, tag="sk")
        nc.sync.dma_start(sk_s, sk_d)
        if use_bias:
            bq_s = const.tile([1, D], bf16, tag="bq")
            nc.sync.dma_start(bq_s, bq_d)
            bk_s = const.tile([1, D], bf16, tag="bk")
            nc.sync.dma_start(bk_s, bk_d)
            bv_s = const.tile([1, D], bf16, tag="bv")
            nc.sync.dma_start(bv_s, bv_d)
            bo_s = const.tile([1, D], bf16, tag="bo")
            nc.sync.dma_start(bo_s, bo_d)
            ones512 = const.tile([1, BLK], bf16, tag="ones512")
            nc.vector.memset(ones512, 1.0)
            onesb = const.tile([1, P], bf16, tag="onesb")
            nc.vector.memset(onesb, 1.0)

        def proj(ps, w_tiles, osl, rhs_s, t_lo, use_b, b_s, oc):
            for dc in range(DC):
                nc.tensor.matmul(ps, w_tiles[dc][:, osl],
                                 rhs_s[dc][:, t_lo:t_lo + BLK],
                                 start=(dc == 0),
                                 stop=(dc == DC - 1 and not use_b))
            if use_b:
                nc.tensor.matmul(ps, b_s[:, oc * P:(oc + 1) * P], ones512,
                                 start=False, stop=True)

        def rope(fin, oc, t_c, cos_s, sin_s):
            # rotate fin[:, oc, t_c*BLK:(t_c+1)*BLK] in place (one producer)
            sl = slice(t_c * BLK, (t_c + 1) * BLK)
            sw = rpool.tile([P, BLK], bf16, tag="sw")
            for (a, src) in ((0, 32), (32, 0), (64, 96), (96, 64)):
                nc.gpsimd.dma_start(sw[a:a + 32, :], fin[src:src + 32, oc, sl])
            t1 = rpool.tile([P, BLK], bf16, tag="t1")
            t2 = rpool.tile([P, BLK], bf16, tag="t2")
            nc.vector.tensor_mul(t1, fin[:, oc, sl], cos_s[:, sl])
            nc.vector.tensor_mul(t2, sw, sin_s[:, sl])
            nc.vector.tensor_add(fin[:, oc, sl], t1, t2)

        # ---- Q/K projections + RoPE
        for oc in range(OC):
            osl = slice(oc * P, (oc + 1) * P)
            for t_c in range(NQ // BLK):
                ps = psmm.tile([P, BLK], f32, tag="mm")
                proj(ps, wq_s, osl, xq_s, t_c * BLK, use_bias,
                     bq_s if use_bias else None, oc)
                nc.any.tensor_copy(qfin[:, oc, t_c * BLK:(t_c + 1) * BLK], ps)
                rope(qfin, oc, t_c, cq_s, sq_s)
            for t_c in range(T // BLK):
                ps = psmm.tile([P, BLK], f32, tag="mm")
                proj(ps, wk_s, osl, x_s, t_c * BLK, use_bias,
                     bk_s if use_bias else None, oc)
                nc.any.tensor_copy(kfin[:, oc, t_c * BLK:(t_c + 1) * BLK], ps)
                rope(kfin, oc, t_c, ck_s, sk_s)

        # ---- V projection (token-major, straight into vaug)
        for tt in range(16):
            for oc2 in range(2):
                ps = psmm.tile([P, BLK], f32, tag="mm")
                for dc in range(DC):
                    nc.tensor.matmul(ps, x_s[dc][:, tt * P:(tt + 1) * P],
                                     wv_s[dc][:, oc2 * BLK:(oc2 + 1) * BLK],
                                     start=(dc == 0),
                                     stop=(dc == DC - 1 and not use_bias))
                if use_bias:
                    nc.tensor.matmul(ps, onesb,
                                     bv_s[:, oc2 * BLK:(oc2 + 1) * BLK],
                                     start=False, stop=True)
                nc.any.tensor_copy(
                    vaug[:, tt, oc2 * 8:(oc2 + 1) * 8, 0:64], ps)

        # ---- phase 2: close projection pools, open attention pools
        ph1_stack.close()
        att_stack = ExitStack()
        psst = att_stack.enter_context(
            tc.tile_pool(name="psst", bufs=1, space="PSUM"))
        pso = att_stack.enter_context(
            tc.tile_pool(name="pso", bufs=2, space="PSUM"))
        ph2 = ctx.enter_context(tc.tile_pool(name="ph2", bufs=1))
        ptp = ctx.enter_context(tc.tile_pool(name="ptp", bufs=2))
        rbp = ctx.enter_context(tc.tile_pool(name="rbp", bufs=2))
        outp = ctx.enter_context(tc.tile_pool(name="outp", bufs=3))

        mA_s = ph2.tile([P, JA, BLK], bf16, tag="mA")
        nc.sync.dma_start(mA_s, mA_d.rearrange("jt p i -> p jt i"))
        mB_s = ph2.tile([P, 8, BLK], bf16, tag="mB")
        nc.sync.dma_start(mB_s, mB_d.rearrange("jt p i -> p jt i"))
        den_sb = ph2.tile([32, BLK], f32, tag="den")
        den_r = ph2.tile([32, BLK], f32, tag="denr")
        ctxu = ph2.tile([P, OC, NQ], bf16, tag="ctxu")

        # ---- attention
        for oc in range(OC):
            h0, h1 = 2 * oc, 2 * oc + 1
            for blk in range(2):
                J = JA if blk == 0 else JB
                q_lo = blk * BLK
                opsA = pso.tile([P, BLK], f32, tag="oA")
                opsB = pso.tile([P, BLK], f32, tag="oB")
                for g in range(J // 2):
                    sA = psst.tile([P, 2 * BLK], f32, tag="sA")
                    sB = psst.tile([P, 2 * BLK], f32, tag="sB")
                    for dj in range(2):
                        jt = 2 * g + dj
                        nc.tensor.matmul(
                            sA[:, dj * BLK:(dj + 1) * BLK],
                            kfin[0:64, oc, jt * P:(jt + 1) * P],
                            qfin[0:64, oc, q_lo:q_lo + BLK],
                            start=True, stop=True, tile_position=(0, 0))
                        nc.tensor.matmul(
                            sB[:, dj * BLK:(dj + 1) * BLK],
                            kfin[64:128, oc, jt * P:(jt + 1) * P],
                            qfin[64:128, oc, q_lo:q_lo + BLK],
                            start=True, stop=True, tile_position=(64, 0))
                    pA = ptp.tile([P, 2 * BLK], bf16, tag="pA")
                    pB = ptp.tile([P, 2 * BLK], bf16, tag="pB")
                    nc.scalar.activation(pA, sA, Exp, scale=0.125)
                    nc.scalar.activation(pB, sB, Exp, scale=0.125)
                    for dj in range(2):
                        jt = 2 * g + dj
                        msk = None
                        if blk == 0:
                            msk = mA_s[:, jt, :]
                        elif jt >= 8:
                            msk = mB_s[:, jt - 8, :]
                        sl = slice(dj * BLK, (dj + 1) * BLK)
                        if msk is not None:
                            nc.vector.tensor_mul(pA[:, sl], pA[:, sl], msk)
                            nc.vector.tensor_mul(pB[:, sl], pB[:, sl], msk)
                        nc.tensor.matmul(opsA[0:65, :], vaug[:, jt, h0, :],
                                         pA[:, sl], start=(jt == 0),
                                         stop=(jt == J - 1))
                        nc.tensor.matmul(opsB[0:65, :], vaug[:, jt, h1, :],
                                         pB[:, sl], start=(jt == 0),
                                         stop=(jt == J - 1))
                row = oc * 4 + blk * 2
                nc.vector.tensor_copy(ctxu[0:64, oc, q_lo:q_lo + BLK],
                                       opsA[0:64, :])
                nc.vector.tensor_copy(ctxu[64:128, oc, q_lo:q_lo + BLK],
                                      opsB[0:64, :])
                for (r, ops) in ((row, opsA), (row + 1, opsB)):
                    stg = rbp.tile([1, BLK], f32, tag="dstage")
                    nc.vector.tensor_copy(stg, ops[64:65, :])
                    nc.gpsimd.dma_start(den_sb[r:r + 1, :], stg)

        # ---- normalize: 1/den broadcast via DRAM round-trip
        nc.vector.reciprocal(den_r, den_sb)
        nc.sync.dma_start(den_d.ap(), den_r)
        for oc in range(OC):
            for blk in range(2):
                row = oc * 4 + blk * 2
                q_lo = blk * BLK
                rb = rbp.tile([P, BLK], f32, tag="rb")
                for (hh, r) in ((0, row), (64, row + 1)):
                    sl = den_d.ap()[r:r + 1, :]
                    src = bass.AP(tensor=sl.tensor, offset=sl.offset,
                                  ap=[[0, 64]] + sl.ap[1:])
                    nc.gpsimd.dma_start(rb[hh:hh + 64, :], src)
                nc.vector.tensor_mul(ctxu[:, oc, q_lo:q_lo + BLK],
                                     ctxu[:, oc, q_lo:q_lo + BLK], rb)

        # ---- output projection
        att_stack.close()
        psmm = ctx.enter_context(
            tc.tile_pool(name="psmm2", bufs=3, space="PSUM"))
        wo_s = []
        for dc in range(DC):
            t = ph2.tile([P, D], bf16, tag=f"wo_s{dc}")
            nc.sync.dma_start(t, WoT[dc * P:(dc + 1) * P, :])
            wo_s.append(t)
        for tcp in range(8):
            for oc2 in range(2):
                ps = psmm.tile([P, BLK], f32, tag="mm")
                for dc in range(DC):
                    nc.tensor.matmul(ps, ctxu[:, dc, tcp * P:(tcp + 1) * P],
                                     wo_s[dc][:, oc2 * BLK:(oc2 + 1) * BLK],
                                     start=(dc == 0),
                                     stop=(dc == DC - 1 and not use_bias))
                if use_bias:
                    nc.tensor.matmul(ps, onesb,
                                     bo_s[:, oc2 * BLK:(oc2 + 1) * BLK],
                                     start=False, stop=True)
                ot = outp.tile([P, BLK], f32, tag="ot")
                nc.any.tensor_copy(ot, ps)
                nc.sync.dma_start(
                    out_d[tcp * P:(tcp + 1) * P,
                          oc2 * BLK:(oc2 + 1) * BLK], ot)
    _legalize_waits(nc)
    return nc


# ------------------------------------------------------------------- entry

def kernel(x, Wq, bq, Wk, bk, Wv, bv, Wo, bo):
    x = np.asarray(x, np.float32)
    Wq, bq = np.asarray(Wq, np.float32), np.asarray(bq, np.float32)
    Wk, bk = np.asarray(Wk, np.float32), np.asarray(bk, np.float32)
    Wv, bv = np.asarray(Wv, np.float32), np.asarray(bv, np.float32)
    Wo, bo = np.asarray(Wo, np.float32), np.asarray(bo, np.float32)
    use_bias = bool(any(np.any(b) for b in (bq, bk, bv, bo)))
    in_maps = host_prep(x, Wq, bq, Wk, bk, Wv, bv, Wo, bo)
    if not use_bias:
        for m in in_maps:
            for k in ("bq", "bk", "bv", "bo"):
                m.pop(k)
    nc = build_nc(use_bias)
    res = run_bass_kernel_spmd(nc, in_maps, list(range(NCORES))).results
    return assemble(res)



# revision 8
# speedup vs baseline: 1.6681x; 1.6681x over previous
# Multi-head causal self-attention with RoPE on 8 NeuronCores (Trainium2).
#
# Sharding: zero-communication batch x head-half split. Core c handles batch
# b = c//2 and heads [8*(c%2) .. 8*(c%2)+8) over ALL 2048 queries. The O
# projection is computed against the core's 512 ctx dims only (row-split
# Wo), producing a partial [T, D] output; the host sums each batch's two
# partials (the "all-reduce" of the hint, done host-side for free).
#
# Why this beats query-split: K/V projections are no longer duplicated
# across the pair, and causal blocks carry no fully-masked j-tiles
# (blocks of 512 queries need exactly 4,8,12,16 j-tiles) - PE work drops
# ~25%. All 8 cores run the same program (SPMD), only data differs.
#
# Engine plan (per core):
#   PE     : projections + scores + AV + O proj  (~590k out-rows, the
#            roofline; kept continuously busy so the 2.4 GHz p-state holds -
#            Q/K projections of later head-pairs are interleaved into the
#            attention loop as "absorber" work that fills dependency stalls)
#   Scalar : exclusively exp() activations (the second-longest engine)
#   Vector : RoPE multiplies, causal-mask multiplies, ctx copies, recip
#   GpSimd : PSUM->SBUF copies, RoPE swap DMAs, denominator staging+norm
#
# Layouts (on chip, bf16 compute / f32 accumulate):
#   qfin/kfin [128 part = head-pair dims, hp, tokens]  d-major for S^T
#   vaug      [128 part = tokens, tt, head, 65]        +ones col -> denom
#   S^T tiles [128 j-tokens, 512 queries]              softmax along PARTITION
#   RoPE "evens-then-odds" head layout baked into W columns host-side so the
#   rotation partner is a fixed +-32 partition shift (4 small swap DMAs).

import sys

import numpy as np
import ml_dtypes

for _p in ("/opt/trn_rl_repo",):
    try:
        import concourse.bass  # noqa: F401
        break
    except ImportError:
        sys.path.insert(0, _p)

import concourse.bass as bass
import concourse.tile as tile
from concourse import mybir
from concourse.bass_utils import run_bass_kernel_spmd

B, T, D, H, DH = 4, 2048, 1024, 16, 64
THETA = 10000.0
NCORES = 8
P = 128
NH = 8      # heads per core
HPC = 4     # head-pairs per core
DC = 8      # 128-wide input-dim chunks
BLK = 512   # query block width
NBLK = 4    # query blocks (J tiles per block: 4,8,12,16)

f32 = mybir.dt.float32
bf16 = mybir.dt.bfloat16
BF = ml_dtypes.bfloat16


# ---------------------------------------------------------------- host prep

def _cols(half):
    """W column order for this core's 8 heads: per head, evens then odds."""
    cols = []
    for h in range(half * NH, (half + 1) * NH):
        b0 = h * DH
        cols += [b0 + 2 * m for m in range(32)]
        cols += [b0 + 2 * m + 1 for m in range(32)]
    return np.asarray(cols)


def _rope_tables():
    """cos/sin [128, T] for the evens-first layout; sin sign baked in."""
    inv = THETA ** (-(np.arange(0, DH, 2, dtype=np.float64) / DH))  # [32]
    m = np.arange(P) % 64
    fi = m % 32
    ang = np.arange(T, dtype=np.float64)[None, :] * inv[fi][:, None]
    cos = np.cos(ang)
    sin = np.sin(ang) * np.where(m < 32, -1.0, 1.0)[:, None]
    return cos.astype(np.float32), sin.astype(np.float32)


def _dmask():
    """Diagonal masks [128, 4, 512]: d-th tile of any block's last 4."""
    jj = np.arange(P)[:, None]
    qq = np.arange(BLK)[None, :]
    return np.stack(
        [(d * P + jj <= qq) for d in range(4)], axis=1).astype(np.float32)


def host_prep(x, Wq, bq, Wk, bk, Wv, bv, Wo, bo):
    cos, sin = _rope_tables()
    dm = _dmask()
    in_maps = []
    for c in range(NCORES):
        b, half = c // 2, c % 2
        cols = _cols(half)
        nat = np.arange(half * 512, (half + 1) * 512)
        wqk = np.concatenate([Wq.T[:, cols], Wk.T[:, cols]], axis=1)
        in_maps.append({
            "xT": np.ascontiguousarray(x[b].T).astype(BF),
            "wqk": np.ascontiguousarray(wqk).astype(BF),
            "wv": np.ascontiguousarray(Wv.T[:, nat]).astype(BF),
            "wo": np.ascontiguousarray(Wo.T[nat, :]).astype(BF),
            "cs": cos.astype(BF), "sn": sin.astype(BF),
            "dm": dm.astype(BF),
            "bqk": np.concatenate([bq[cols], bk[cols]]).reshape(1, D).astype(BF),
            "bv": bv[nat].reshape(1, 512).astype(BF),
            "bo2": (bo / 2).reshape(1, D).astype(BF),  # halved: partials sum
        })
    return in_maps


def assemble(results):
    y = np.empty((B, T, D), np.float32)
    for b in range(B):
        y[b] = (results[2 * b]["out"].astype(np.float32)
                + results[2 * b + 1]["out"].astype(np.float32))
    return y


# ------------------------------------------------------------- device build

def _legalize_waits(nc, max_waits=1):
    """Limit every instruction to one sync-wait command (walrus encoding)."""
    eng_obj = {
        mybir.EngineType.PE: nc.tensor,
        mybir.EngineType.Activation: nc.scalar,
        mybir.EngineType.DVE: nc.vector,
        mybir.EngineType.Pool: nc.gpsimd,
        mybir.EngineType.SP: nc.sync,
    }
    fn = nc.m.functions[0]
    for blk in fn.blocks:
        insts = list(blk.instructions)
        new = []
        for inst in insts:
            si = inst.sync_info
            nw = len(si.on_wait) if si is not None else 0
            if nw > max_waits:
                for w in si.on_wait[: nw - max_waits]:
                    eng_obj[inst.engine].nop()
                    nop = fn.blocks[-1].instructions[-1]
                    fn.blocks[-1].instructions = \
                        fn.blocks[-1].instructions[:-1]
                    nop.sync_info = mybir.SyncInfo(on_wait=[w], on_update=[])
                    new.append(nop)
                inst.sync_info = mybir.SyncInfo(
                    on_wait=list(si.on_wait[nw - max_waits:]),
                    on_update=list(si.on_update))
            new.append(inst)
        blk.instructions = new


def build_nc(use_bias):
    from contextlib import ExitStack

    nc = bass.Bass("TRN2", target_bir_lowering=False, debug=False,
                   num_devices=NCORES)
    Exp = mybir.ActivationFunctionType.Exp

    xT = nc.dram_tensor("xT", [D, T], bf16, kind="ExternalInput").ap()
    wqk_d = nc.dram_tensor("wqk", [D, D], bf16, kind="ExternalInput").ap()
    wv_d = nc.dram_tensor("wv", [D, 512], bf16, kind="ExternalInput").ap()
    wo_d = nc.dram_tensor("wo", [512, D], bf16, kind="ExternalInput").ap()
    cs_d = nc.dram_tensor("cs", [P, T], bf16, kind="ExternalInput").ap()
    sn_d = nc.dram_tensor("sn", [P, T], bf16, kind="ExternalInput").ap()
    dm_d = nc.dram_tensor("dm", [P, 4, BLK], bf16, kind="ExternalInput").ap()
    if use_bias:
        bqk_d = nc.dram_tensor("bqk", [1, D], bf16, kind="ExternalInput").ap()
        bv_d = nc.dram_tensor("bv", [1, 512], bf16, kind="ExternalInput").ap()
        bo2_d = nc.dram_tensor("bo2", [1, D], bf16, kind="ExternalInput").ap()
    out_d = nc.dram_tensor("out", [T, D], bf16, kind="ExternalOutput").ap()
    den_d = nc.dram_tensor("den_scratch", [32, BLK], f32)

    with tile.TileContext(nc) as tc, ExitStack() as ctx:
        const = ctx.enter_context(tc.tile_pool(name="const", bufs=1))
        rpool = ctx.enter_context(tc.tile_pool(name="rp", bufs=2))
        ptp = ctx.enter_context(tc.tile_pool(name="ptp", bufs=3))
        obuf = ctx.enter_context(tc.tile_pool(name="ob", bufs=3))
        rbp = ctx.enter_context(tc.tile_pool(name="rbp", bufs=2))
        stgp = ctx.enter_context(tc.tile_pool(name="stg", bufs=4))
        # PSUM: one static layout for the whole kernel (8 banks total):
        #   psst 2 bufs x [128,2,512] f32, one tag (A/B alternate
        #        generations; scores)                            -> 4
        #   pso  1 buf  x 2x[128,512] f32 (AV accumulators)      -> 2
        #   projp 2 bufs x [128,512] f32 (projections + O proj)  -> 2
        projp = ctx.enter_context(tc.tile_pool(name="pj", bufs=2, space="PSUM"))
        psst = ctx.enter_context(tc.tile_pool(name="ps", bufs=2, space="PSUM"))
        pso = ctx.enter_context(tc.tile_pool(name="po", bufs=1, space="PSUM"))

        # ---- persistent SBUF tensors
        x_s, wqk_s, wv_s, wo_s = [], [], [], []
        issuers = [nc.sync, nc.scalar, nc.gpsimd]
        for dc in range(DC):
            t = const.tile([P, D], bf16, tag=f"wqk{dc}")
            issuers[dc % 3].dma_start(t, wqk_d[dc * P:(dc + 1) * P, :])
            wqk_s.append(t)
        for dc in range(DC):
            t = const.tile([P, T], bf16, tag=f"x{dc}")
            issuers[dc % 3].dma_start(t, xT[dc * P:(dc + 1) * P, :])
            x_s.append(t)
        for dc in range(DC):
            t = const.tile([P, 512], bf16, tag=f"wv{dc}")
            issuers[dc % 3].dma_start(t, wv_d[dc * P:(dc + 1) * P, :])
            wv_s.append(t)
        cs_s = const.tile([P, T], bf16, tag="cs")
        nc.sync.dma_start(cs_s, cs_d)
        sn_s = const.tile([P, T], bf16, tag="sn")
        nc.scalar.dma_start(sn_s, sn_d)
        mask_s = const.tile([P, 4, BLK], bf16, tag="dm")
        nc.gpsimd.dma_start(mask_s, dm_d)
        for dc in range(4):
            t = const.tile([P, D], bf16, tag=f"wo{dc}")
            issuers[dc % 3].dma_start(t, wo_d[dc * P:(dc + 1) * P, :])
            wo_s.append(t)
        if use_bias:
            bqk_s = const.tile([1, D], bf16, tag="bqk")
            nc.sync.dma_start(bqk_s, bqk_d)
            bv_s = const.tile([1, 512], bf16, tag="bv")
            nc.sync.dma_start(bv_s, bv_d)
            bo2_s = const.tile([1, D], bf16, tag="bo2")
            nc.sync.dma_start(bo2_s, bo2_d)
            ones512 = const.tile([1, BLK], bf16, tag="ones512")
            nc.vector.memset(ones512, 1.0)
            onesb = const.tile([1, P], bf16, tag="onesb")
            nc.vector.memset(onesb, 1.0)

        qfin = const.tile([P, HPC, T], bf16, tag="qfin")
        kfin = const.tile([P, HPC, T], bf16, tag="kfin")
        vaug = const.tile([P, 16, NH, 65], bf16, tag="vaug")
        nc.vector.memset(vaug[:, :, :, 64:65], 1.0)
        ctxu = const.tile([P, HPC, T], bf16, tag="ctxu")
        den_sb, den_r = [], []
        for hp in range(HPC):
            dtile = const.tile([8, BLK], f32, tag=f"den{hp}")
            den_sb.append(dtile)
            rtile = const.tile([8, BLK], f32, tag=f"denr{hp}")
            den_r.append(rtile)

        # ---- emission helpers -------------------------------------------
        def _copy(eng, dst, src_):
            if eng is nc.scalar:
                nc.scalar.copy(dst, src_)
            else:
                eng.tensor_copy(dst, src_)

        def qk_chunk(kind, hp, tcb, copy_eng):
            """Project one [128 dims, 512 tok] chunk of q (kind=0) or
            k (kind=1) for head-pair hp, then RoPE it in place."""
            oc = kind * 4 + hp
            fin = qfin if kind == 0 else kfin
            ps = projp.tile([P, BLK], f32, tag="pj")
            for dc in range(DC):
                nc.tensor.matmul(ps, wqk_s[dc][:, oc * P:(oc + 1) * P],
                                 x_s[dc][:, tcb * BLK:(tcb + 1) * BLK],
                                 start=(dc == 0),
                                 stop=(dc == DC - 1 and not use_bias))
            if use_bias:
                nc.tensor.matmul(ps, bqk_s[:, oc * P:(oc + 1) * P], ones512,
                                 start=False, stop=True)
            sl = slice(tcb * BLK, (tcb + 1) * BLK)
            dst = fin[:, hp, sl]
            _copy(copy_eng, dst, ps)
            sw = rpool.tile([P, BLK], bf16, tag="sw")
            for (a, src) in ((0, 32), (32, 0), (64, 96), (96, 64)):
                nc.gpsimd.dma_start(sw[a:a + 32, :], fin[src:src + 32, hp, sl])
            t1 = rpool.tile([P, BLK], bf16, tag="t1")
            t2 = rpool.tile([P, BLK], bf16, tag="t2")
            nc.vector.tensor_mul(t1, dst, cs_s[:, sl])
            nc.vector.tensor_mul(t2, sw, sn_s[:, sl])
            nc.vector.tensor_add(dst, t1, t2)

        def v_chunk(tt, copy_eng):
            """Project V for one 128-token tile (token-major into vaug)."""
            ps = projp.tile([P, BLK], f32, tag="pj")
            for dc in range(DC):
                nc.tensor.matmul(ps, x_s[dc][:, tt * P:(tt + 1) * P],
                                 wv_s[dc],
                                 start=(dc == 0),
                                 stop=(dc == DC - 1 and not use_bias))
            if use_bias:
                nc.tensor.matmul(ps, onesb, bv_s, start=False, stop=True)
            _copy(copy_eng, vaug[:, tt, :, 0:64], ps)

        def o_chunk(tcp, oc2):
            """O-projection for one [128 tok, 512 out] tile + store."""
            ps = projp.tile([P, BLK], f32, tag="pj")
            for dc in range(HPC):
                nc.tensor.matmul(ps, ctxu[:, dc, tcp * P:(tcp + 1) * P],
                                 wo_s[dc][:, oc2 * BLK:(oc2 + 1) * BLK],
                                 start=(dc == 0),
                                 stop=(dc == HPC - 1 and not use_bias))
            if use_bias:
                nc.tensor.matmul(ps, onesb,
                                 bo2_s[:, oc2 * BLK:(oc2 + 1) * BLK],
                                 start=False, stop=True)
            ot = obuf.tile([P, BLK], bf16, tag="ot")
            nc.vector.tensor_copy(ot, ps)
            nc.sync.dma_start(
                out_d[tcp * P:(tcp + 1) * P,
                      oc2 * BLK:(oc2 + 1) * BLK], ot)

        def norm_blk(hp, blk):
            """Broadcast 1/den from DRAM and scale ctx for one block."""
            r = (hp * 4 + blk) * 2
            q_lo = blk * BLK
            rb = rbp.tile([P, BLK], f32, tag="rb")
            for (hh, rr) in ((0, r), (64, r + 1)):
                sl_ = den_d.ap()[rr:rr + 1, :]
                src = bass.AP(tensor=sl_.tensor, offset=sl_.offset,
                              ap=[[0, 64]] + sl_.ap[1:])
                nc.gpsimd.dma_start(rb[hh:hh + 64, :], src)
            nc.gpsimd.tensor_mul(ctxu[:, hp, q_lo:q_lo + BLK],
                                 ctxu[:, hp, q_lo:q_lo + BLK], rb)

        # ---- phase 1: Q/K for hp0, V for tt0..7 --------------------------
        for tcb in range(4):
            qk_chunk(0, 0, tcb, nc.scalar)
        for tcb in range(4):
            qk_chunk(1, 0, tcb, nc.scalar)
        for tt in range(8):
            v_chunk(tt, nc.scalar)

        # absorber: independent PE work drained inside the attention loop
        work = []
        for tt in range(8, 16):
            work.append(lambda tt=tt: v_chunk(tt, nc.vector))
        for hp in range(1, HPC):
            for tcb in range(4):
                work.append(
                    lambda hp=hp, tcb=tcb: qk_chunk(0, hp, tcb, nc.vector))
            for tcb in range(4):
                work.append(
                    lambda hp=hp, tcb=tcb: qk_chunk(1, hp, tcb, nc.vector))
        # drain target before global group g (piecewise-linear, deadlines:
        # V by g8, QK hp1 by g18, hp2 by g38, hp3 by g58)
        knots = [(0, 0), (8, 8), (18, 16), (38, 24), (58, 32), (80, 32)]

        def target(g):
            for (g0, n0), (g1, n1) in zip(knots, knots[1:]):
                if g <= g1:
                    return min(32, int(np.ceil(
                        n0 + (n1 - n0) * (g - g0) / max(1, g1 - g0))))
            return 32

        drained = [0]

        def drain_to(n):
            while drained[0] < min(n, len(work)):
                work[drained[0]]()
                drained[0] += 1

        # ---- attention ---------------------------------------------------
        # Software-pipelined: AV of group g-1 is emitted after the scores
        # and exp of group g, so the PE never waits on the exp->mask chain;
        # absorber chunks drain between scores and AV to fill the
        # scalar-vs-PE pacing gap.
        g_global = [0]
        for hp in range(HPC):
            for blk in range(NBLK):
                J = 4 * (blk + 1)
                q_lo = blk * BLK
                opsA = pso.tile([P, BLK], f32, tag="oA")
                opsB = pso.tile([P, BLK], f32, tag="oB")
                pend = None  # (pA, pB, g) awaiting AV emission
                for g in range(J // 2):
                    sA = psst.tile([P, 2, BLK], f32, tag="s")
                    for dj in range(2):
                        jt = 2 * g + dj
                        nc.tensor.matmul(
                            sA[:, dj, :],
                            kfin[0:64, hp, jt * P:(jt + 1) * P],
                            qfin[0:64, hp, q_lo:q_lo + BLK],
                            start=True, stop=True, tile_position=(0, 0))
                    sB = psst.tile([P, 2, BLK], f32, tag="s")
                    for dj in range(2):
                        jt = 2 * g + dj
                        nc.tensor.matmul(
                            sB[:, dj, :],
                            kfin[64:128, hp, jt * P:(jt + 1) * P],
                            qfin[64:128, hp, q_lo:q_lo + BLK],
                            start=True, stop=True, tile_position=(64, 0))
                    pA = ptp.tile([P, 2, BLK], bf16, tag="pA")
                    pB = ptp.tile([P, 2, BLK], bf16, tag="pB")
                    nc.scalar.activation(pA, sA, Exp, scale=0.125)
                    nc.scalar.activation(pB, sB, Exp, scale=0.125)
                    for dj in range(2):
                        jt = 2 * g + dj
                        d = jt - (J - 4)
                        if d >= 0:
                            nc.vector.tensor_mul(pA[:, dj, :], pA[:, dj, :],
                                                 mask_s[:, d, :])
                            nc.vector.tensor_mul(pB[:, dj, :], pB[:, dj, :],
                                                 mask_s[:, d, :])
                    drain_to(target(g_global[0]))
                    g_global[0] += 1

                    def emit_av(pA, pB, g):
                        for dj in range(2):
                            jt = 2 * g + dj
                            nc.tensor.matmul(opsA[0:65, :],
                                             vaug[:, jt, 2 * hp, :],
                                             pA[:, dj, :], start=(jt == 0),
                                             stop=(jt == J - 1))
                            nc.tensor.matmul(opsB[0:65, :],
                                             vaug[:, jt, 2 * hp + 1, :],
                                             pB[:, dj, :], start=(jt == 0),
                                             stop=(jt == J - 1))
                    if pend is not None:
                        emit_av(*pend)
                    pend = (pA, pB, g)
                emit_av(*pend)
                # ctx + denominator staging
                nc.vector.tensor_copy(ctxu[0:64, hp, q_lo:q_lo + BLK],
                                      opsA[0:64, :])
                nc.vector.tensor_copy(ctxu[64:128, hp, q_lo:q_lo + BLK],
                                      opsB[0:64, :])
                r = blk * 2
                for (rr, ops) in ((r, opsA), (r + 1, opsB)):
                    stg = stgp.tile([1, BLK], f32, tag="dstage")
                    nc.vector.tensor_copy(stg, ops[64:65, :])
                    nc.gpsimd.dma_start(den_sb[hp][rr:rr + 1, :], stg)
            # per-hp: reciprocal + DRAM round-trip + normalize
            r0 = hp * 8
            nc.vector.reciprocal(den_r[hp], den_sb[hp])
            nc.sync.dma_start(den_d.ap()[r0:r0 + 8, :], den_r[hp])
            for blk in range(NBLK):
                norm_blk(hp, blk)
        drain_to(len(work))

        # ---- O projection ------------------------------------------------
        for tcp in range(16):
            for oc2 in range(2):
                o_chunk(tcp, oc2)

    _legalize_waits(nc)
    return nc


# ------------------------------------------------------------------- entry

def kernel(x, Wq, bq, Wk, bk, Wv, bv, Wo, bo):
    x = np.asarray(x, np.float32)
    Wq, bq = np.asarray(Wq, np.float32), np.asarray(bq, np.float32)
    Wk, bk = np.asarray(Wk, np.float32), np.asarray(bk, np.float32)
    Wv, bv = np.asarray(Wv, np.float32), np.asarray(bv, np.float32)
    Wo, bo = np.asarray(Wo, np.float32), np.asarray(bo, np.float32)
    use_bias = bool(any(np.any(b) for b in (bq, bk, bv, bo)))
    in_maps = host_prep(x, Wq, bq, Wk, bk, Wv, bv, Wo, bo)
    if not use_bias:
        for m in in_maps:
            for k in ("bqk", "bv", "bo2"):
                m.pop(k)
    nc = build_nc(use_bias)
    res = run_bass_kernel_spmd(nc, in_maps, list(range(NCORES))).results
    return assemble(res)


# revision 11
# speedup vs baseline: 1.7561x; 1.0528x over previous
# Multi-head causal self-attention with RoPE on 8 NeuronCores (Trainium2).
#
# Sharding: zero-communication batch x head-half split. Core c handles batch
# b = c//2 and heads [8*(c%2) .. 8*(c%2)+8) over ALL 2048 queries. The O
# projection is computed against the core's 512 ctx dims only (row-split
# Wo), producing a partial [T, D] output; the host sums each batch's two
# partials (the "all-reduce" of the hint, done host-side for free).
#
# Why this beats query-split: K/V projections are no longer duplicated
# across the pair, and causal blocks carry no fully-masked j-tiles
# (blocks of 512 queries need exactly 4,8,12,16 j-tiles) - PE work drops
# ~25%. All 8 cores run the same program (SPMD), only data differs.
#
# Engine plan (per core):
#   PE     : projections + scores + AV + O proj  (~590k out-rows, the
#            roofline; kept continuously busy so the 2.4 GHz p-state holds -
#            Q/K projections of later head-pairs are interleaved into the
#            attention loop as "absorber" work that fills dependency stalls)
#   Scalar : exclusively exp() activations (the second-longest engine)
#   Vector : RoPE multiplies, causal-mask multiplies, ctx copies, recip
#   GpSimd : PSUM->SBUF copies, RoPE swap DMAs, denominator staging+norm
#
# Layouts (on chip, bf16 compute / f32 accumulate):
#   qfin/kfin [128 part = head-pair dims, hp, tokens]  d-major for S^T
#   vaug      [128 part = tokens, tt, head, 65]        +ones col -> denom
#   S^T tiles [128 j-tokens, 512 queries]              softmax along PARTITION
#   RoPE "evens-then-odds" head layout baked into W columns host-side so the
#   rotation partner is a fixed +-32 partition shift (4 small swap DMAs).

import sys

import numpy as np
import ml_dtypes

for _p in ("/opt/trn_rl_repo",):
    try:
        import concourse.bass  # noqa: F401
        break
    except ImportError:
        sys.path.insert(0, _p)

import concourse.bass as bass
import concourse.tile as tile
from concourse import mybir
from concourse.bass_utils import run_bass_kernel_spmd

B, T, D, H, DH = 4, 2048, 1024, 16, 64
THETA = 10000.0
NCORES = 8
P = 128
NH = 8      # heads per core
HPC = 4     # head-pairs per core
DC = 8      # 128-wide input-dim chunks
BLK = 512   # query block width
NBLK = 4    # query blocks (J tiles per block: 4,8,12,16)

f32 = mybir.dt.float32
bf16 = mybir.dt.bfloat16
BF = ml_dtypes.bfloat16


# ---------------------------------------------------------------- host prep

def _cols(half):
    """W column order for this core's 8 heads: per head, evens then odds."""
    cols = []
    for h in range(half * NH, (half + 1) * NH):
        b0 = h * DH
        cols += [b0 + 2 * m for m in range(32)]
        cols += [b0 + 2 * m + 1 for m in range(32)]
    return np.asarray(cols)


def _rope_tables():
    """cos/sin [128, T] for the evens-first layout; sin sign baked in."""
    inv = THETA ** (-(np.arange(0, DH, 2, dtype=np.float64) / DH))  # [32]
    m = np.arange(P) % 64
    fi = m % 32
    ang = np.arange(T, dtype=np.float64)[None, :] * inv[fi][:, None]
    cos = np.cos(ang)
    sin = np.sin(ang) * np.where(m < 32, -1.0, 1.0)[:, None]
    return cos.astype(np.float32), sin.astype(np.float32)


def _dmask():
    """Diagonal masks [128, 4, 512]: d-th tile of any block's last 4."""
    jj = np.arange(P)[:, None]
    qq = np.arange(BLK)[None, :]
    return np.stack(
        [(d * P + jj <= qq) for d in range(4)], axis=1).astype(np.float32)


def host_prep(x, Wq, bq, Wk, bk, Wv, bv, Wo, bo):
    cos, sin = _rope_tables()
    dm = _dmask()
    in_maps = []
    for c in range(NCORES):
        b, half = c // 2, c % 2
        cols = _cols(half)
        nat = np.arange(half * 512, (half + 1) * 512)
        wqk = np.concatenate([Wq.T[:, cols], Wk.T[:, cols]], axis=1)
        in_maps.append({
            "xT": np.ascontiguousarray(x[b].T).astype(BF),
            "wqk": np.ascontiguousarray(wqk).astype(BF),
            "wv": np.ascontiguousarray(Wv.T[:, nat]).astype(BF),
            "wo": np.ascontiguousarray(Wo.T[nat, :]).astype(BF),
            "cs": cos.astype(BF), "sn": sin.astype(BF),
            "dm": dm.astype(BF),
            "ind2": np.stack([
                (np.arange(P) < 64), (np.arange(P) >= 64)]).astype(np.float32),
            "bqk": np.concatenate([bq[cols], bk[cols]]).reshape(1, D).astype(BF),
            "bv": bv[nat].reshape(1, 512).astype(BF),
            "bo2": (bo / 2).reshape(1, D).astype(BF),  # halved: partials sum
        })
    return in_maps


def assemble(results):
    y = np.empty((B, T, D), np.float32)
    for b in range(B):
        y[b] = (results[2 * b]["out"].astype(np.float32)
                + results[2 * b + 1]["out"].astype(np.float32))
    return y


# ------------------------------------------------------------- device build

def _legalize_waits(nc, max_waits=1):
    """Limit every instruction to one sync-wait command (walrus encoding)."""
    eng_obj = {
        mybir.EngineType.PE: nc.tensor,
        mybir.EngineType.Activation: nc.scalar,
        mybir.EngineType.DVE: nc.vector,
        mybir.EngineType.Pool: nc.gpsimd,
        mybir.EngineType.SP: nc.sync,
    }
    fn = nc.m.functions[0]
    for blk in fn.blocks:
        insts = list(blk.instructions)
        new = []
        for inst in insts:
            si = inst.sync_info
            nw = len(si.on_wait) if si is not None else 0
            if nw > max_waits:
                for w in si.on_wait[: nw - max_waits]:
                    eng_obj[inst.engine].nop()
                    nop = fn.blocks[-1].instructions[-1]
                    fn.blocks[-1].instructions = \
                        fn.blocks[-1].instructions[:-1]
                    nop.sync_info = mybir.SyncInfo(on_wait=[w], on_update=[])
                    new.append(nop)
                inst.sync_info = mybir.SyncInfo(
                    on_wait=list(si.on_wait[nw - max_waits:]),
                    on_update=list(si.on_update))
            new.append(inst)
        blk.instructions = new


def build_nc(use_bias):
    from contextlib import ExitStack

    nc = bass.Bass("TRN2", target_bir_lowering=False, debug=False,
                   num_devices=NCORES)
    Exp = mybir.ActivationFunctionType.Exp

    xT = nc.dram_tensor("xT", [D, T], bf16, kind="ExternalInput").ap()
    wqk_d = nc.dram_tensor("wqk", [D, D], bf16, kind="ExternalInput").ap()
    wv_d = nc.dram_tensor("wv", [D, 512], bf16, kind="ExternalInput").ap()
    wo_d = nc.dram_tensor("wo", [512, D], bf16, kind="ExternalInput").ap()
    cs_d = nc.dram_tensor("cs", [P, T], bf16, kind="ExternalInput").ap()
    sn_d = nc.dram_tensor("sn", [P, T], bf16, kind="ExternalInput").ap()
    dm_d = nc.dram_tensor("dm", [P, 4, BLK], bf16, kind="ExternalInput").ap()
    ind_d = nc.dram_tensor("ind2", [2, P], f32, kind="ExternalInput").ap()
    if use_bias:
        bqk_d = nc.dram_tensor("bqk", [1, D], bf16, kind="ExternalInput").ap()
        bv_d = nc.dram_tensor("bv", [1, 512], bf16, kind="ExternalInput").ap()
        bo2_d = nc.dram_tensor("bo2", [1, D], bf16, kind="ExternalInput").ap()
    out_d = nc.dram_tensor("out", [T, D], bf16, kind="ExternalOutput").ap()
    den_d = nc.dram_tensor("den_scratch", [32, BLK], f32)

    with tile.TileContext(nc) as tc, ExitStack() as ctx:
        const = ctx.enter_context(tc.tile_pool(name="const", bufs=1))
        rpool = ctx.enter_context(tc.tile_pool(name="rp", bufs=2))
        ptp = ctx.enter_context(tc.tile_pool(name="ptp", bufs=3))
        obuf = ctx.enter_context(tc.tile_pool(name="ob", bufs=3))
        rbp = ctx.enter_context(tc.tile_pool(name="rbp", bufs=2))
        stgp = ctx.enter_context(tc.tile_pool(name="stg", bufs=4))
        denp = ctx.enter_context(tc.tile_pool(name="denp", bufs=3))
        # PSUM: one static layout for the whole kernel (8 banks total):
        #   psst 2 bufs x [128,2,512] f32, one tag (A/B alternate
        #        generations; scores)                            -> 4
        #   pso  1 buf  x 2x[128,512] f32 (AV accumulators)      -> 2
        #   projp 2 bufs x [128,512] f32 (projections + O proj)  -> 2
        projp = ctx.enter_context(tc.tile_pool(name="pj", bufs=2, space="PSUM"))
        psst = ctx.enter_context(tc.tile_pool(name="ps", bufs=2, space="PSUM"))
        pso = ctx.enter_context(tc.tile_pool(name="po", bufs=1, space="PSUM"))

        # ---- persistent SBUF tensors
        x_s, wqk_s, wv_s, wo_s = [], [], [], []
        issuers = [nc.sync, nc.scalar, nc.gpsimd]
        for dc in range(DC):
            t = const.tile([P, D], bf16, tag=f"wqk{dc}")
            issuers[dc % 3].dma_start(t, wqk_d[dc * P:(dc + 1) * P, :])
            wqk_s.append(t)
        for dc in range(DC):
            t = const.tile([P, T], bf16, tag=f"x{dc}")
            issuers[dc % 3].dma_start(t, xT[dc * P:(dc + 1) * P, :])
            x_s.append(t)
        for dc in range(DC):
            t = const.tile([P, 512], bf16, tag=f"wv{dc}")
            issuers[dc % 3].dma_start(t, wv_d[dc * P:(dc + 1) * P, :])
            wv_s.append(t)
        cs_s = const.tile([P, T], bf16, tag="cs")
        nc.sync.dma_start(cs_s, cs_d)
        sn_s = const.tile([P, T], bf16, tag="sn")
        nc.scalar.dma_start(sn_s, sn_d)
        mask_s = const.tile([P, 4, BLK], bf16, tag="dm")
        nc.gpsimd.dma_start(mask_s, dm_d)
        for dc in range(4):
            t = const.tile([P, D], bf16, tag=f"wo{dc}")
            issuers[dc % 3].dma_start(t, wo_d[dc * P:(dc + 1) * P, :])
            wo_s.append(t)
        if use_bias:
            bqk_s = const.tile([1, D], bf16, tag="bqk")
            nc.sync.dma_start(bqk_s, bqk_d)
            bv_s = const.tile([1, 512], bf16, tag="bv")
            nc.sync.dma_start(bv_s, bv_d)
            bo2_s = const.tile([1, D], bf16, tag="bo2")
            nc.sync.dma_start(bo2_s, bo2_d)
            ones512 = const.tile([1, BLK], bf16, tag="ones512")
            nc.vector.memset(ones512, 1.0)
            onesb = const.tile([1, P], bf16, tag="onesb")
            nc.vector.memset(onesb, 1.0)

        qfin = const.tile([P, HPC, T], bf16, tag="qfin")
        kfin = const.tile([P, HPC, T], bf16, tag="kfin")
        vaug = const.tile([P, 16, NH, 65], bf16, tag="vaug")
        nc.vector.memset(vaug[:, :, :, 64:65], 1.0)
        ctxu = const.tile([P, HPC, T], bf16, tag="ctxu")
        den_sb, den_r = [], []
        for hp in range(HPC):
            dtile = const.tile([8, BLK], f32, tag=f"den{hp}")
            den_sb.append(dtile)
            rtile = const.tile([8, BLK], f32, tag=f"denr{hp}")
            den_r.append(rtile)

        # ---- emission helpers -------------------------------------------
        def _copy(eng, dst, src_):
            if eng is nc.scalar:
                nc.scalar.copy(dst, src_)
            else:
                eng.tensor_copy(dst, src_)

        def qk_chunk(kind, hp, tcb, copy_eng):
            """Project one [128 dims, 512 tok] chunk of q (kind=0) or
            k (kind=1) for head-pair hp, then RoPE it in place."""
            oc = kind * 4 + hp
            fin = qfin if kind == 0 else kfin
            ps = projp.tile([P, BLK], f32, tag="pj")
            for dc in range(DC):
                nc.tensor.matmul(ps, wqk_s[dc][:, oc * P:(oc + 1) * P],
                                 x_s[dc][:, tcb * BLK:(tcb + 1) * BLK],
                                 start=(dc == 0),
                                 stop=(dc == DC - 1 and not use_bias))
            if use_bias:
                nc.tensor.matmul(ps, bqk_s[:, oc * P:(oc + 1) * P], ones512,
                                 start=False, stop=True)
            sl = slice(tcb * BLK, (tcb + 1) * BLK)
            dst = fin[:, hp, sl]
            _copy(copy_eng, dst, ps)
            sw = rpool.tile([P, BLK], bf16, tag="sw")
            for (a, src) in ((0, 32), (32, 0), (64, 96), (96, 64)):
                nc.gpsimd.dma_start(sw[a:a + 32, :], fin[src:src + 32, hp, sl])
            t1 = rpool.tile([P, BLK], bf16, tag="t1")
            t2 = rpool.tile([P, BLK], bf16, tag="t2")
            nc.vector.tensor_mul(t1, dst, cs_s[:, sl])
            nc.vector.tensor_mul(t2, sw, sn_s[:, sl])
            nc.vector.tensor_add(dst, t1, t2)

        def v_chunk(tt, copy_eng):
            """Project V for one 128-token tile (token-major into vaug)."""
            ps = projp.tile([P, BLK], f32, tag="pj")
            for dc in range(DC):
                nc.tensor.matmul(ps, x_s[dc][:, tt * P:(tt + 1) * P],
                                 wv_s[dc],
                                 start=(dc == 0),
                                 stop=(dc == DC - 1 and not use_bias))
            if use_bias:
                nc.tensor.matmul(ps, onesb, bv_s, start=False, stop=True)
            _copy(copy_eng, vaug[:, tt, :, 0:64], ps)

        def o_chunk(tcp, oc2):
            """O-projection for one [128 tok, 512 out] tile + store."""
            ps = projp.tile([P, BLK], f32, tag="pj")
            for dc in range(HPC):
                nc.tensor.matmul(ps, ctxu[:, dc, tcp * P:(tcp + 1) * P],
                                 wo_s[dc][:, oc2 * BLK:(oc2 + 1) * BLK],
                                 start=(dc == 0),
                                 stop=(dc == HPC - 1 and not use_bias))
            if use_bias:
                nc.tensor.matmul(ps, onesb,
                                 bo2_s[:, oc2 * BLK:(oc2 + 1) * BLK],
                                 start=False, stop=True)
            ot = obuf.tile([P, BLK], bf16, tag="ot")
            nc.vector.tensor_copy(ot, ps)
            nc.sync.dma_start(
                out_d[tcp * P:(tcp + 1) * P,
                      oc2 * BLK:(oc2 + 1) * BLK], ot)

        def norm_blk(hp, blk):
            """Broadcast 1/den from DRAM and scale ctx for one block."""
            r = (hp * 4 + blk) * 2
            q_lo = blk * BLK
            rb = rbp.tile([P, BLK], f32, tag="rb")
            for (hh, rr) in ((0, r), (64, r + 1)):
                sl_ = den_d.ap()[rr:rr + 1, :]
                src = bass.AP(tensor=sl_.tensor, offset=sl_.offset,
                              ap=[[0, 64]] + sl_.ap[1:])
                nc.gpsimd.dma_start(rb[hh:hh + 64, :], src)
            nc.gpsimd.tensor_mul(ctxu[:, hp, q_lo:q_lo + BLK],
                                 ctxu[:, hp, q_lo:q_lo + BLK], rb)

        # ---- phase 1: Q/K for hp0, V for tt0..7 --------------------------
        for tcb in range(4):
            qk_chunk(0, 0, tcb, nc.scalar)
        for tcb in range(4):
            qk_chunk(1, 0, tcb, nc.scalar)
        for tt in range(8):
            v_chunk(tt, nc.scalar)

        # absorber: independent PE work drained inside the attention loop
        work = []
        for tt in range(8, 16):
            work.append(lambda tt=tt: v_chunk(tt, nc.vector))
        for hp in range(1, HPC):
            for tcb in range(4):
                work.append(
                    lambda hp=hp, tcb=tcb: qk_chunk(0, hp, tcb, nc.vector))
            for tcb in range(4):
                work.append(
                    lambda hp=hp, tcb=tcb: qk_chunk(1, hp, tcb, nc.vector))
        # drain target before global group g (piecewise-linear, deadlines:
        # V by g8, QK hp1 by g18, hp2 by g38, hp3 by g58)
        knots = [(0, 0), (8, 8), (18, 16), (38, 24), (58, 32), (80, 32)]

        def target(g):
            for (g0, n0), (g1, n1) in zip(knots, knots[1:]):
                if g <= g1:
                    return min(32, int(np.ceil(
                        n0 + (n1 - n0) * (g - g0) / max(1, g1 - g0))))
            return 32

        drained = [0]

        def drain_to(n):
            while drained[0] < min(n, len(work)):
                work[drained[0]]()
                drained[0] += 1

        # ---- attention ---------------------------------------------------
        # Software-pipelined: AV of group g-1 is emitted after the scores
        # and exp of group g, so the PE never waits on the exp->mask chain;
        # absorber chunks drain between scores and AV to fill the
        # scalar-vs-PE pacing gap.
        g_global = [0]
        for hp in range(HPC):
            for blk in range(NBLK):
                J = 4 * (blk + 1)
                q_lo = blk * BLK
                opsA = pso.tile([P, BLK], f32, tag="oA")
                opsB = pso.tile([P, BLK], f32, tag="oB")
                pend = None  # (pA, pB, g) awaiting AV emission
                for g in range(J // 2):
                    # diagonal j-tiles only cover queries >= 128*d; trim
                    # the matmul N-range (the masked region's stale PSUM
                    # gets exp'd but then zeroed by the mask multiply)
                    lo = [P * max(0, 2 * g + dj - (J - 4)) for dj in (0, 1)]
                    sA = psst.tile([P, 2, BLK], f32, tag="s")
                    for dj in range(2):
                        jt = 2 * g + dj
                        nc.tensor.matmul(
                            sA[:, dj, lo[dj]:],
                            kfin[0:64, hp, jt * P:(jt + 1) * P],
                            qfin[0:64, hp, q_lo + lo[dj]:q_lo + BLK],
                            start=True, stop=True, tile_position=(0, 0))
                    sB = psst.tile([P, 2, BLK], f32, tag="s")
                    for dj in range(2):
                        jt = 2 * g + dj
                        nc.tensor.matmul(
                            sB[:, dj, lo[dj]:],
                            kfin[64:128, hp, jt * P:(jt + 1) * P],
                            qfin[64:128, hp, q_lo + lo[dj]:q_lo + BLK],
                            start=True, stop=True, tile_position=(64, 0))
                    pA = ptp.tile([P, 2, BLK], bf16, tag="pA")
                    pB = ptp.tile([P, 2, BLK], bf16, tag="pB")
                    nc.scalar.activation(pA, sA, Exp, scale=0.125)
                    nc.scalar.activation(pB, sB, Exp, scale=0.125)
                    for dj in range(2):
                        jt = 2 * g + dj
                        d = jt - (J - 4)
                        if d >= 0:
                            nc.vector.tensor_mul(pA[:, dj, :], pA[:, dj, :],
                                                 mask_s[:, d, :])
                            nc.vector.tensor_mul(pB[:, dj, :], pB[:, dj, :],
                                                 mask_s[:, d, :])
                    drain_to(target(g_global[0]))
                    g_global[0] += 1

                    def emit_av(pA, pB, g):
                        for dj in range(2):
                            jt = 2 * g + dj
                            lo = P * max(0, jt - (J - 4))
                            nc.tensor.matmul(opsA[0:65, lo:],
                                             vaug[:, jt, 2 * hp, :],
                                             pA[:, dj, lo:], start=(jt == 0),
                                             stop=(jt == J - 1))
                            nc.tensor.matmul(opsB[0:65, lo:],
                                             vaug[:, jt, 2 * hp + 1, :],
                                             pB[:, dj, lo:], start=(jt == 0),
                                             stop=(jt == J - 1))
                    if pend is not None:
                        emit_av(*pend)
                    pend = (pA, pB, g)
                emit_av(*pend)
                # ctx + denominator staging
                nc.vector.tensor_copy(ctxu[0:64, hp, q_lo:q_lo + BLK],
                                      opsA[0:64, :])
                nc.vector.tensor_copy(ctxu[64:128, hp, q_lo:q_lo + BLK],
                                      opsB[0:64, :])
                r = blk * 2
                for (rr, ops) in ((r, opsA), (r + 1, opsB)):
                    stg = stgp.tile([1, BLK], f32, tag="dstage")
                    nc.vector.tensor_copy(stg, ops[64:65, :])
                    nc.gpsimd.dma_start(den_sb[hp][rr:rr + 1, :], stg)
            # per-hp: reciprocal + DRAM round-trip + normalize
            r0 = hp * 8
            nc.vector.reciprocal(den_r[hp], den_sb[hp])
            nc.sync.dma_start(den_d.ap()[r0:r0 + 8, :], den_r[hp])
            for blk in range(NBLK):
                norm_blk(hp, blk)
        drain_to(len(work))

        # ---- O projection ------------------------------------------------
        for tcp in range(16):
            for oc2 in range(2):
                o_chunk(tcp, oc2)

    _legalize_waits(nc)
    return nc


# ------------------------------------------------------------------- entry

def kernel(x, Wq, bq, Wk, bk, Wv, bv, Wo, bo):
    x = np.asarray(x, np.float32)
    Wq, bq = np.asarray(Wq, np.float32), np.asarray(bq, np.float32)
    Wk, bk = np.asarray(Wk, np.float32), np.asarray(bk, np.float32)
    Wv, bv = np.asarray(Wv, np.float32), np.asarray(bv, np.float32)
    Wo, bo = np.asarray(Wo, np.float32), np.asarray(bo, np.float32)
    use_bias = bool(any(np.any(b) for b in (bq, bk, bv, bo)))
    in_maps = host_prep(x, Wq, bq, Wk, bk, Wv, bv, Wo, bo)
    if not use_bias:
        for m in in_maps:
            for k in ("bqk", "bv", "bo2"):
                m.pop(k)
    nc = build_nc(use_bias)
    res = run_bass_kernel_spmd(nc, in_maps, list(range(NCORES))).results
    return assemble(res)
